# revision 4
# baseline (speedup 1.0000x reference)
"""Distance-discriminator kernel for 8 Trainium2 cores (V5, bf16).

Math (reference): for x [N, D],
    sq[i,d] = sum_j (x[j,d]-x[i,d])^2 = Q_d - 2*S_d*x + N*x^2,
    S_d = sum_j x[j,d],  Q_d = sum_j x[j,d]^2,  m_d = S_d/N
    out = log(sqrt(sq) + eps) @ W.T + b  with eps negligible (dist ~ sqrt(2N)).

Device mapping (columns d sharded 512/core, x shipped bf16 - rel err ~4e-4
measured against a 2e-2 gate; all accumulators fp32):
  1. m_d: pairwise column-fold tree on DVE (tensor_tensor add at 2x bf16:
     4096->2048->1024) then two 512-wide bn_stats + aggregate (~3.6us/chunk
     vs 5.8 for direct 8-segment bn_stats; bn is 1x-locked by hardware).
  2. quadratic pass, split to balance the engines (bn/AMR are DVE-only and
     Ln is ACT-only, so this is the only movable pass):
     - chunks 0,1 on ACT: u = Square(x - m), accum_out -> C = sum u free;
       ln arg is u*N + C.
     - chunks 2,3 on DVE: y = (x - 2m)*x via one affine_mul_reduce (1x but
       single-pass), accum -> A = Q - 2N*m^2; ln arg is y*N + Q.
  3. logd2c = Ln(arg * e^-C0) via per-partition scale/bias (centered by C0 so
     bf16 GEMM inputs carry fluctuation, not the ~8.9 mean; C0 folded into
     the host bias).
  4. out.T partial = (W/2)^T @ logd2c via bf16 matmuls into 8 PSUM banks,
     evacuated DVE/ACT, summed across cores on host (a device-side collective
     costs ~50us first-use on this stack), bias + C0 correction added on host.
  x streams on all three DMA queues: chunk 0 whole on the gpsimd SWDGE queue
  (its descriptor gen starts before the HWDGE queues' first issue), chunks
  1-3 as halves on sync/scalar.
"""

import numpy as np
import ml_dtypes

import concourse.bacc as bacc
import concourse.bass as bass
import concourse.tile as tile
from concourse import mybir
from concourse.bass_utils import run_bass_kernel_spmd

N = 4096          # rows
D = 4096          # feature columns
OUT = 64
NCORES = 8
DC = D // NCORES  # 512 columns per core
KCH = DC // 128   # 4 partition-chunks per core
C0 = 8.9          # ln(sq) centering constant; absorbed via host bias
EMC0 = float(np.exp(-C0))
ACT_SQ = (0, 1)   # chunks whose quadratic runs on ACT (rest on DVE)

F32 = mybir.dt.float32
BF16 = mybir.dt.bfloat16
_cache: dict = {}


def _build():
    nc = bacc.Bacc(
        "TRN2",
        target_bir_lowering=False,
        debug=False,
        num_devices=NCORES,
    )
    xT = nc.dram_tensor("xT", [DC, N], BF16, kind="ExternalInput").ap()
    wT = nc.dram_tensor("wT", [128, KCH * OUT], BF16, kind="ExternalInput").ap()
    out = nc.dram_tensor("out", [OUT, N], F32, kind="ExternalOutput").ap()

    AL = mybir.AluOpType
    with tile.TileContext(nc) as tc:
        with (
            tc.tile_pool(name="wp", bufs=1) as wp,
            tc.tile_pool(name="xp", bufs=KCH) as xp,
            tc.tile_pool(name="zp", bufs=2) as zp,
            tc.tile_pool(name="st", bufs=KCH) as st,
            tc.tile_pool(name="up", bufs=2) as up,
            tc.tile_pool(name="lp", bufs=2) as lp,
            tc.tile_pool(name="pp", bufs=8, space="PSUM") as pp,
        ):
            xs = []
            for k in range(KCH):
                x_k = xp.tile([128, N], BF16, name=f"x_{k}", tag="x")
                if k == 0:
                    nc.gpsimd.dma_start(x_k[:], xT[k * 128 : (k + 1) * 128, :])
                else:
                    nc.sync.dma_start(
                        x_k[:, : N // 2], xT[k * 128 : (k + 1) * 128, : N // 2]
                    )
                    nc.scalar.dma_start(
                        x_k[:, N // 2 :], xT[k * 128 : (k + 1) * 128, N // 2 :]
                    )
                xs.append(x_k)
            w_all = wp.tile([128, KCH * OUT], BF16, name="w_all", tag="w_all")
            nc.scalar.dma_start(w_all[:], wT)
            # preload the Ln table set while ACT idles during the x stream
            # (the Square set loads implicitly at the first square)
            scr = wp.tile([128, 1], BF16, name="scr", tag="scr")
            nc.scalar.activation(
                scr[:], w_all[:, 0:1], mybir.ActivationFunctionType.Ln,
                bias=w_all[:, 1:2], scale=1.0,
            )

            out_sb = wp.tile([OUT, N], F32, name="out_sb", tag="out_sb")
            psums = [pp.tile([OUT, 512], F32, name=f"ps_{j}", tag="ps") for j in range(8)]

            for k in range(KCH):
                x_k = xs[k]
                # stats: column-fold tree 4096 -> 2048 -> 1024 (bf16 adds at
                # 2x), then bn_stats on the partial sums; mean(t2) = S/1024.
                # High priority: stats gate everything else for the chunk.
                with tc.high_priority():
                    t1_k = zp.tile([128, N // 2], BF16, name=f"t1_{k}", tag="t1")
                    nc.vector.tensor_tensor(
                        t1_k[:], x_k[:, : N // 2], x_k[:, N // 2 :], op=AL.add
                    )
                    t2_k = zp.tile([128, N // 4], BF16, name=f"t2_{k}", tag="t2")
                    nc.vector.tensor_tensor(
                        t2_k[:], t1_k[:, : N // 4], t1_k[:, N // 4 :], op=AL.add
                    )
                    stats_k = st.tile([128, 2, 6], F32, name=f"stats_{k}", tag="stats")
                    nc.vector.bn_stats(stats_k[:, 0, :], t2_k[:, 0:512])
                    nc.vector.bn_stats(stats_k[:, 1, :], t2_k[:, 512:1024])
                    mv_k = st.tile([128, 2], F32, name=f"mv_{k}", tag="mv")
                    nc.vector.bn_aggr(mv_k[:], stats_k[:])
                    nm_k = st.tile([128, 2], F32, name=f"nm_{k}", tag="nm")
                    # [:,0] = -m = -mean(t2)/4 (ACT path); [:,1] = -2m (DVE path)
                    nc.vector.tensor_scalar_mul(nm_k[:, 0:1], mv_k[:, 0:1], -0.25)
                    nc.vector.tensor_scalar_mul(nm_k[:, 1:2], mv_k[:, 0:1], -0.5)

                u_k = up.tile([128, N], BF16, name=f"u_{k}", tag="u")
                C_k = st.tile([128, 2], F32, name=f"C_{k}", tag="C")
                bC_k = st.tile([128, 1], F32, name=f"bC_{k}", tag="bC")
                if k in ACT_SQ:
                    # u = (x - m)^2 ; accum C = sum u ; ln bias = C*e^-C0
                    nc.scalar.activation(
                        u_k[:], x_k[:], mybir.ActivationFunctionType.Square,
                        bias=nm_k[:, 0:1], scale=1.0, accum_out=C_k[:, 0:1],
                    )
                    nc.vector.tensor_scalar_mul(bC_k[:], C_k[:, 0:1], EMC0)
                else:
                    # y = (x - 2m)*x in one AMR pass; accum A = Q - 2N*m^2
                    # halves so the tail chunk's Ln can start after half A
                    for h in range(2):
                        nc.vector.affine_mul_reduce(
                            u_k[:, h * (N // 2) : (h + 1) * (N // 2)],
                            C_k[:, h : h + 1],
                            x_k[:, h * (N // 2) : (h + 1) * (N // 2)],
                            x_k[:, h * (N // 2) : (h + 1) * (N // 2)],
                            scale=1.0, bias=nm_k[:, 1:2],
                        )
                    # ln bias = Q*e^-C0 = (A + 2N*m^2)*e^-C0
                    m2_k = st.tile([128, 1], F32, name=f"m2_{k}", tag="m2")
                    nc.vector.tensor_tensor(
                        m2_k[:], mv_k[:, 0:1], mv_k[:, 0:1], op=AL.mult
                    )
                    qa_k = st.tile([128, 1], F32, name=f"qa_{k}", tag="qa")
                    nc.vector.tensor_scalar(
                        qa_k[:], C_k[:, 0:1], C_k[:, 1:2], None, op0=AL.add
                    )
                    # m2 holds (S/1024)^2 = 16*m^2 -> 2N*m^2 = m2 * N/8
                    nc.vector.tensor_scalar(
                        bC_k[:], m2_k[:], float(N) / 8.0, qa_k[:],
                        op0=AL.mult, op1=AL.add,
                    )
                    nc.vector.tensor_scalar_mul(bC_k[:], bC_k[:], EMC0)

                l_k = lp.tile([128, N], BF16, name=f"l_{k}", tag="l")
                npiece = 4 if k == KCH - 1 else 1
                wq = N // npiece
                for q in range(npiece):
                    nc.scalar.activation(
                        l_k[:, q * wq : (q + 1) * wq],
                        u_k[:, q * wq : (q + 1) * wq],
                        mybir.ActivationFunctionType.Ln,
                        bias=bC_k[:], scale=float(N) * EMC0,
                    )
                    nb = wq // 512
                    for jj in range(nb):
                        j = q * nb + jj
                        nc.tensor.matmul(
                            psums[j][:],
                            lhsT=w_all[:, k * OUT : (k + 1) * OUT],
                            rhs=l_k[:, j * 512 : (j + 1) * 512],
                            start=(k == 0),
                            stop=(k == KCH - 1),
                        )

            # evacuate PSUM (no bias - host adds it); out DMA per pair of banks
            for j in range(8):
                if j % 2 == 0:
                    nc.vector.tensor_copy(out_sb[:, j * 512 : (j + 1) * 512], psums[j][:])
                else:
                    nc.scalar.copy(out_sb[:, j * 512 : (j + 1) * 512], psums[j][:])
                if j % 2 == 1:
                    nc.sync.dma_start(
                        out[:, (j - 1) * 512 : (j + 1) * 512],
                        out_sb[:, (j - 1) * 512 : (j + 1) * 512],
                    )

    nc.compile()
    return nc


def _prep_inputs(data, W, b):
    data = np.asarray(data, dtype=np.float32)
    W = np.asarray(W, dtype=np.float32)
    b = np.asarray(b, dtype=np.float32)
    xb = data.astype(ml_dtypes.bfloat16)               # [N, D] bf16
    w2T = (0.5 * W).T.astype(ml_dtypes.bfloat16)       # [D, OUT] bf16
    in_maps = []
    for c in range(NCORES):
        xT_c = np.ascontiguousarray(xb[:, c * DC : (c + 1) * DC].T)   # [DC, N]
        w_c = (
            w2T[c * DC : (c + 1) * DC, :]
            .reshape(KCH, 128, OUT)
            .transpose(1, 0, 2)
            .reshape(128, KCH * OUT)
        )
        in_maps.append({"xT": xT_c, "wT": np.ascontiguousarray(w_c)})
    host_bias = (b + C0 * (0.5 * W).sum(axis=1)).astype(np.float32)   # [OUT]
    return in_maps, host_bias


def _run(inputs, trace=False, **kwargs):
    if "nc" not in _cache:
        _cache["nc"] = _build()
    nc = _cache["nc"]
    in_maps, host_bias = _prep_inputs(inputs["data"], inputs["W"], inputs["b"])
    res = run_bass_kernel_spmd(
        nc, in_maps, core_ids=list(range(NCORES)), trace=trace, **kwargs
    )
    outT = np.sum([res.results[c]["out"] for c in range(NCORES)], axis=0, dtype=np.float32)
    return np.ascontiguousarray(outT.T + host_bias[None, :]), res


def kernel(data, W, b):
    out, _ = _run({"data": data, "W": W, "b": b})
    return out


# revision 5
# speedup vs baseline: 1.0725x; 1.0725x over previous
"""Distance-discriminator kernel for 8 Trainium2 cores (V6, bf16).

Math (reference): for x [N, D],
    sq[i,d] = sum_j (x[j,d]-x[i,d])^2 = Q_d - 2*S_d*x + N*x^2
            = N*(x - m_d)^2 + C_d,   m_d = S_d/N, C_d = Q_d - S_d^2/N
    out = log(sqrt(sq) + eps) @ W.T + b  with eps negligible (dist ~ sqrt(2N)).

Device mapping (columns d sharded 512/core, x shipped bf16 - rel err ~4e-4
measured against a 2e-2 gate; all accumulators fp32). Per 128-partition
chunk, picked to keep ACT and DVE equally busy (bn/AMR are DVE-only, Ln is
ACT-only, the quadratic pass can go either way):
  chunk 0,1: mean via DVE column-fold tree (tensor_tensor add at 2x bf16,
      4096->2048->1024, then two bn_stats); u = Square(x - m) on ACT with
      accum_out -> C free; Ln bias = C*e^-C0.
  chunk 2:   mean via fold tree; y = (x - 2m)*x in one DVE affine_mul_reduce
      (accum A -> Q = A + 2N*m^2); Ln(y*N + Q) form.
  chunk 3:   direct 8-segment bn_stats whose segments chase the three DMA
      pieces as they land (gives var -> C with no accumulation dependency),
      then v = x - m (tensor_scalar 4x) and u = v*v (tensor_tensor 2x) in
      column quarters, so the tail chunk's Ln pipeline drains fast.
  logd2c = Ln(arg*N*e^-C0 + bias) centered by C0 so bf16 GEMM inputs carry
  fluctuation, not the ~8.9 mean (C0 folded into the host bias). GEMM:
  out.T partial = (W/2)^T @ logd2c, bf16 matmuls into 8 PSUM banks,
  evacuated DVE/ACT, partials summed on host (a device-side collective
  costs ~50us first-use on this stack), bias + C0 correction added on host.
  x streams chunk-major across all three DMA queues (sync/scalar HWDGE +
  gpsimd SWDGE) so chunk k lands at ~2.5*(k+1)us.
"""

import numpy as np
import ml_dtypes

import concourse.bacc as bacc
import concourse.bass as bass
import concourse.tile as tile
from concourse import mybir
from concourse.bass_utils import run_bass_kernel_spmd

N = 4096          # rows
D = 4096          # feature columns
OUT = 64
NCORES = 8
DC = D // NCORES  # 512 columns per core
KCH = DC // 128   # 4 partition-chunks per core
C0 = 8.9          # ln(sq) centering constant; absorbed via host bias
EMC0 = float(np.exp(-C0))
SLN = float(N) * EMC0

F32 = mybir.dt.float32
BF16 = mybir.dt.bfloat16
_cache: dict = {}


def _build():
    nc = bacc.Bacc(
        "TRN2",
        target_bir_lowering=False,
        debug=False,
        num_devices=NCORES,
    )
    xT = nc.dram_tensor("xT", [DC, N], BF16, kind="ExternalInput").ap()
    wT = nc.dram_tensor("wT", [128, KCH * OUT], BF16, kind="ExternalInput").ap()
    out = nc.dram_tensor("out", [OUT, N], F32, kind="ExternalOutput").ap()

    AL = mybir.AluOpType
    ACT = mybir.ActivationFunctionType
    with tile.TileContext(nc) as tc:
        with (
            tc.tile_pool(name="wp", bufs=1) as wp,
            tc.tile_pool(name="xp", bufs=KCH) as xp,
            tc.tile_pool(name="zp", bufs=2) as zp,
            tc.tile_pool(name="st", bufs=KCH) as st,
            tc.tile_pool(name="up", bufs=2) as up,
            tc.tile_pool(name="lp", bufs=2) as lp,
            tc.tile_pool(name="pp", bufs=8, space="PSUM") as pp,
        ):
            # --- DMA: w first (tiny), then x chunk-major on all 3 queues ---
            w_all = wp.tile([128, KCH * OUT], BF16, name="w_all", tag="w_all")
            nc.scalar.dma_start(w_all[:], wT)
            xs = []
            for k in range(KCH):
                x_k = xp.tile([128, N], BF16, name=f"x_{k}", tag="x")
                r = slice(k * 128, (k + 1) * 128)
                nc.sync.dma_start(x_k[:, 0:1536], xT[r, 0:1536])
                nc.scalar.dma_start(x_k[:, 1536:3072], xT[r, 1536:3072])
                nc.gpsimd.dma_start(x_k[:, 3072:4096], xT[r, 3072:4096])
                xs.append(x_k)
            # preload the Ln table set while ACT idles during the x stream
            # (the Square set loads implicitly at the first square)
            scr = wp.tile([128, 1], BF16, name="scr", tag="scr")
            nc.scalar.activation(scr[:], w_all[:, 0:1], ACT.Ln,
                                 bias=w_all[:, 1:2], scale=1.0)

            out_sb = wp.tile([OUT, N], F32, name="out_sb", tag="out_sb")
            psums = [pp.tile([OUT, 512], F32, name=f"ps_{j}", tag="ps")
                     for j in range(8)]

            def fold_stats(k):
                """mean of chunk k via 2-level column-fold + bn_stats.
                Returns mv ([128,2]; [:,0] = mean(t2) = S/1024)."""
                x_k = xs[k]
                t1 = zp.tile([128, N // 2], BF16, name=f"t1_{k}", tag="t1")
                nc.vector.tensor_tensor(t1[:], x_k[:, : N // 2], x_k[:, N // 2 :], op=AL.add)
                t2 = zp.tile([128, N // 4], BF16, name=f"t2_{k}", tag="t2")
                nc.vector.tensor_tensor(t2[:], t1[:, : N // 4], t1[:, N // 4 :], op=AL.add)
                stt = st.tile([128, 2, 6], F32, name=f"stats_{k}", tag="stats")
                nc.vector.bn_stats(stt[:, 0, :], t2[:, 0:512])
                nc.vector.bn_stats(stt[:, 1, :], t2[:, 512:1024])
                mv = st.tile([128, 2], F32, name=f"mv_{k}", tag="mv")
                nc.vector.bn_aggr(mv[:], stt[:])
                return mv

            def ln_mm(k, l_k, u_in, bC, pieces):
                """Ln piece(s) of chunk k + the bank matmuls behind each."""
                wq = N // pieces
                for q in range(pieces):
                    nc.scalar.activation(
                        l_k[:, q * wq : (q + 1) * wq], u_in[:, q * wq : (q + 1) * wq],
                        ACT.Ln, bias=bC[:], scale=SLN)
                    for jj in range(wq // 512):
                        j = q * (wq // 512) + jj
                        nc.tensor.matmul(
                            psums[j][:], lhsT=w_all[:, k * OUT : (k + 1) * OUT],
                            rhs=l_k[:, j * 512 : (j + 1) * 512],
                            start=(k == 0), stop=(k == KCH - 1))

            # --- chunk 0 (ACT square) ---
            mv0 = fold_stats(0)
            nm0 = st.tile([128, 1], F32, name="nm_0", tag="nm")
            nc.vector.tensor_scalar_mul(nm0[:], mv0[:, 0:1], -0.25)
            u0 = up.tile([128, N], BF16, name="u_0", tag="u")
            C0a = st.tile([128, 1], F32, name="C_0", tag="C")
            nc.scalar.activation(u0[:], xs[0][:], ACT.Square,
                                 bias=nm0[:], scale=1.0, accum_out=C0a[:])

            # --- chunk 1 stats ---
            mv1 = fold_stats(1)
            nm1 = st.tile([128, 1], F32, name="nm_1", tag="nm")
            nc.vector.tensor_scalar_mul(nm1[:], mv1[:, 0:1], -0.25)

            # bC0 (depends on chunk-0 accum) then ln0
            bC0 = st.tile([128, 1], F32, name="bC_0", tag="bC")
            nc.vector.tensor_scalar_mul(bC0[:], C0a[:], EMC0)
            l0 = lp.tile([128, N], BF16, name="l_0", tag="l")
            ln_mm(0, l0, u0, bC0, 1)

            # --- chunk 1 square (ACT) ---
            u1 = up.tile([128, N], BF16, name="u_1", tag="u")
            C1a = st.tile([128, 1], F32, name="C_1", tag="C")
            nc.scalar.activation(u1[:], xs[1][:], ACT.Square,
                                 bias=nm1[:], scale=1.0, accum_out=C1a[:])

            # --- chunk 2 stats + first AMR half ---
            mv2 = fold_stats(2)
            nm2 = st.tile([128, 1], F32, name="nm_2", tag="nm")
            nc.vector.tensor_scalar_mul(nm2[:], mv2[:, 0:1], -0.5)  # -2m
            u2 = up.tile([128, N], BF16, name="u_2", tag="u")
            A2 = st.tile([128, 2], F32, name="C_2", tag="C")
            nc.vector.affine_mul_reduce(
                u2[:, : N // 2], A2[:, 0:1], xs[2][:, : N // 2], xs[2][:, : N // 2],
                scale=1.0, bias=nm2[:])

            # --- chunk 3 direct bn segments chasing its three DMA pieces ---
            st3 = st.tile([128, 8, 6], F32, name="stats_3", tag="stats3")
            for s in range(3):
                nc.vector.bn_stats(st3[:, s, :], xs[3][:, s * 512 : (s + 1) * 512])

            # bC1 (depends on chunk-1 accum) then ln1
            bC1 = st.tile([128, 1], F32, name="bC_1", tag="bC")
            nc.vector.tensor_scalar_mul(bC1[:], C1a[:], EMC0)
            l1 = lp.tile([128, N], BF16, name="l_1", tag="l")
            ln_mm(1, l1, u1, bC1, 1)

            # --- chunk 2 second AMR half + Q fixup ---
            nc.vector.affine_mul_reduce(
                u2[:, N // 2 :], A2[:, 1:2], xs[2][:, N // 2 :], xs[2][:, N // 2 :],
                scale=1.0, bias=nm2[:])
            m22 = st.tile([128, 1], F32, name="m2_2", tag="m2")
            nc.vector.tensor_tensor(m22[:], mv2[:, 0:1], mv2[:, 0:1], op=AL.mult)
            qa2 = st.tile([128, 1], F32, name="qa_2", tag="qa")
            nc.vector.tensor_scalar(qa2[:], A2[:, 0:1], A2[:, 1:2], None, op0=AL.add)
            bC2 = st.tile([128, 1], F32, name="bC_2", tag="bC")
            # mean(t2)^2 = 16 m^2 -> 2N m^2 = m22 * N/8 ; Q = A + 2N m^2
            nc.vector.tensor_scalar(bC2[:], m22[:], float(N) / 8.0, qa2[:],
                                    op0=AL.mult, op1=AL.add)
            nc.vector.tensor_scalar_mul(bC2[:], bC2[:], EMC0)

            # rest of chunk 3 bn segments + stats-derived bias (var -> C)
            for s in range(3, 8):
                nc.vector.bn_stats(st3[:, s, :], xs[3][:, s * 512 : (s + 1) * 512])
            mv3 = st.tile([128, 2], F32, name="mv_3", tag="mv")
            nc.vector.bn_aggr(mv3[:], st3[:])
            nm3 = st.tile([128, 1], F32, name="nm_3", tag="nm")
            nc.vector.tensor_scalar_mul(nm3[:], mv3[:, 0:1], -1.0)
            bC3 = st.tile([128, 1], F32, name="bC_3", tag="bC")
            nc.vector.tensor_scalar_mul(bC3[:], mv3[:, 1:2], SLN)  # N*var*e^-C0

            # ln2 behind the chunk-2 fixups
            l2 = lp.tile([128, N], BF16, name="l_2", tag="l")
            ln_mm(2, l2, u2, bC2, 1)

            # chunk 3 quadratic: v = x - m (4x), u = v*v per quarter (2x)
            v3 = up.tile([128, N], BF16, name="v_3", tag="v", bufs=1)
            nc.vector.tensor_scalar(v3[:], xs[3][:], nm3[:], None, op0=AL.add)
            u3 = up.tile([128, N], BF16, name="u_3", tag="u")
            l3 = lp.tile([128, N], BF16, name="l_3", tag="l")
            for q in range(4):
                cs = slice(q * (N // 4), (q + 1) * (N // 4))
                nc.vector.tensor_tensor(u3[:, cs], v3[:, cs], v3[:, cs], op=AL.mult)
                nc.scalar.activation(l3[:, cs], u3[:, cs], ACT.Ln,
                                     bias=bC3[:], scale=SLN)
                for jj in range(2):
                    j = q * 2 + jj
                    nc.tensor.matmul(
                        psums[j][:], lhsT=w_all[:, 3 * OUT : 4 * OUT],
                        rhs=l3[:, j * 512 : (j + 1) * 512],
                        start=False, stop=True)

            # evacuate PSUM (no bias - host adds it); out DMA per pair
            for j in range(8):
                if j < 6:
                    nc.vector.tensor_copy(out_sb[:, j * 512 : (j + 1) * 512], psums[j][:])
                else:
                    nc.scalar.copy(out_sb[:, j * 512 : (j + 1) * 512], psums[j][:])
                if j % 2 == 1:
                    nc.sync.dma_start(
                        out[:, (j - 1) * 512 : (j + 1) * 512],
                        out_sb[:, (j - 1) * 512 : (j + 1) * 512])

    nc.compile()
    return nc


def _prep_inputs(data, W, b):
    data = np.asarray(data, dtype=np.float32)
    W = np.asarray(W, dtype=np.float32)
    b = np.asarray(b, dtype=np.float32)
    xb = data.astype(ml_dtypes.bfloat16)               # [N, D] bf16
    w2T = (0.5 * W).T.astype(ml_dtypes.bfloat16)       # [D, OUT] bf16
    in_maps = []
    for c in range(NCORES):
        xT_c = np.ascontiguousarray(xb[:, c * DC : (c + 1) * DC].T)   # [DC, N]
        w_c = (
            w2T[c * DC : (c + 1) * DC, :]
            .reshape(KCH, 128, OUT)
            .transpose(1, 0, 2)
            .reshape(128, KCH * OUT)
        )
        in_maps.append({"xT": xT_c, "wT": np.ascontiguousarray(w_c)})
    host_bias = (b + C0 * (0.5 * W).sum(axis=1)).astype(np.float32)   # [OUT]
    return in_maps, host_bias


def _run(inputs, trace=False, **kwargs):
    if "nc" not in _cache:
        _cache["nc"] = _build()
    nc = _cache["nc"]
    in_maps, host_bias = _prep_inputs(inputs["data"], inputs["W"], inputs["b"])
    res = run_bass_kernel_spmd(
        nc, in_maps, core_ids=list(range(NCORES)), trace=trace, **kwargs
    )
    outT = np.sum([res.results[c]["out"] for c in range(NCORES)], axis=0, dtype=np.float32)
    return np.ascontiguousarray(outT.T + host_bias[None, :]), res


def kernel(data, W, b):
    out, _ = _run({"data": data, "W": W, "b": b})
    return out
